# revision 1
# baseline (speedup 1.0000x reference)
import math

import jax
import jax.numpy as jnp
import numpy as np

EPS = 1e-9
B, Q, L, NC = 128, 2048, 64, 8
ZI = -4.0 * math.pi * 1e-9  # constant imaginary part of (q/2)^2 - 4*pi*sld_comp


def _abeles_real(q, thickness, roughness, sld):
    # q: (b,Q) f32; thickness, roughness: (b,L); sld: (b,L+1) — all real f32.
    # Complex math done manually in f32 pairs so the neuron compiler never
    # sees complex64.
    amb = sld[:, 0:1]
    c_n = 4.0 * math.pi * (sld - amb) * 1e-6  # (b, L+1)

    q2 = (q * 0.5) ** 2  # (b, Q)
    zr = q2[:, :, None] - c_n[:, None, :]  # (b, Q, L+1)

    # stable complex sqrt of (zr + i*ZI), ZI < 0
    h = jnp.sqrt(zr * zr + ZI * ZI)
    t = jnp.sqrt(0.5 * (h + jnp.abs(zr)))
    w = (0.5 * -ZI) / t
    pos = zr >= 0.0
    kr = jnp.where(pos, t, w)
    ki = jnp.where(pos, -w, -t)

    kcr, kci = kr[..., :-1], ki[..., :-1]  # (b,Q,L)
    knr, kni = kr[..., 1:], ki[..., 1:]

    t_in = thickness[:, None, :]  # (b,1,L)
    rr2 = -2.0 * (roughness * roughness)[:, None, :]  # (b,1,L)

    # exp(i t k) and exp(-i t k)
    tkr = t_in * kcr
    tki = t_in * kci
    eb = jnp.exp(-tki)
    emb = jnp.exp(tki)
    cb = jnp.cos(tkr)
    sb = jnp.sin(tkr)
    ebr, ebi = eb * cb, eb * sb
    embr, embi = emb * cb, -emb * sb

    # fresnel r_n = (kc - kn)/(kc + kn + EPS) * exp(-2 kc kn rough^2)
    nr, ni = kcr - knr, kci - kni
    dr, di = kcr + knr + EPS, kci + kni
    inv = 1.0 / (dr * dr + di * di)
    ratr = (nr * dr + ni * di) * inv
    rati = (ni * dr - nr * di) * inv

    kkr = kcr * knr - kci * kni
    kki = kcr * kni + kci * knr
    wr = rr2 * kkr
    wi = rr2 * kki
    ew = jnp.exp(wr)
    ewr, ewi = ew * jnp.cos(wi), ew * jnp.sin(wi)

    rnr = ratr * ewr - rati * ewi
    rni = ratr * ewi + rati * ewr

    # matrix entries (real/imag planes), each (b,Q,L)
    m00r, m00i = ebr, ebi
    m01r = rnr * ebr - rni * ebi
    m01i = rnr * ebi + rni * ebr
    m10r = rnr * embr - rni * embi
    m10i = rnr * embi + rni * embr
    m11r, m11i = embr, embi

    planes = (m00r, m00i, m01r, m01i, m10r, m10i, m11r, m11i)
    init = tuple(p[..., 0] for p in planes)
    ms = tuple(jnp.moveaxis(p[..., 1:], -1, 0) for p in planes)

    def step(carry, m):
        ar, ai, br, bi, cr, ci, dr_, di_ = carry
        er, ei, fr, fi, gr, gi, hr, hi = m
        nar = ar * er - ai * ei + br * gr - bi * gi
        nai = ar * ei + ai * er + br * gi + bi * gr
        nbr = ar * fr - ai * fi + br * hr - bi * hi
        nbi = ar * fi + ai * fr + br * hi + bi * hr
        ncr = cr * er - ci * ei + dr_ * gr - di_ * gi
        nci = cr * ei + ci * er + dr_ * gi + di_ * gr
        ndr = cr * fr - ci * fi + dr_ * hr - di_ * hi
        ndi = cr * fi + ci * fr + dr_ * hi + di_ * hr
        return (nar, nai, nbr, nbi, ncr, nci, ndr, ndi), None

    (ar, ai, _, _, cr, ci, _, _), _ = jax.lax.scan(step, init, ms)
    ar = ar + EPS
    inv2 = 1.0 / (ar * ar + ai * ai)
    qr = (cr * ar + ci * ai) * inv2
    qi = (ci * ar - cr * ai) * inv2
    return qr * qr + qi * qi


_pmapped = jax.pmap(_abeles_real)


def kernel(q, thickness, roughness, sld):
    qs = q.reshape(NC, B // NC, Q)
    ts = thickness.reshape(NC, B // NC, L)
    rs = roughness.reshape(NC, B // NC, L)
    ss = sld.reshape(NC, B // NC, L + 1)
    out = _pmapped(qs, ts, rs, ss)
    return np.asarray(jax.device_get(out)).reshape(B, Q).astype(np.float32)

